# revision 11
# baseline (speedup 1.0000x reference)
"""Trainium2 Bass kernel for a pre-norm transformer block (nn_Block).

Math (per batch b of x [4, 1024, 1024]):
    h  = LN(x) ; qkv = h @ w_qkv + b_qkv ; attention (16 heads, dh=64)
    x  = x + (attn_out @ w_proj + b_proj)
    h  = LN(x) ; x = x + gelu(h @ w_fc1 + b_fc1) @ w_fc2 + b_fc2

Sharding: communication-free hybrid. Core c handles batch b = c // 2 and
query-token half c % 2. Each core computes K and V for its batch's full
1024 tokens (duplicated across the 2 cores sharing a batch, ~14% extra
flops) and everything else only for its own 512 query tokens. No
collectives.

On-chip layout is feature-major ("transposed"): every activation lives as
[features, tokens] so all linear layers run as out^T = W^T @ in^T with the
weight (as stored, [in, out]) the stationary operand and the activation the
moving operand; no transposes are ever materialized. V is produced
token-major directly by swapping the matmul operand roles. Softmax runs on
S^T = K^T-major scores: exp on ACT (scale folded in), denominators come
from an extra ones-column appended to V (row 64 of the PV product), and
normalization is deferred to the [64, 512] attention output. LayerNorm
statistics are computed with ones-vector matmuls (fp32r, full speed) since
features live on partitions; per-token scale/shift vectors are replicated
across partitions with a broadcast DMA and applied on the vector engine.

Matmul operands are bf16 (fp32 accumulation in PSUM); LN stats, softmax,
residuals stay fp32. Measured vs the fp32 reference: rel_l2 ~1.5e-3.
"""

import os
import sys
import types

import numpy as np

# concourse ships in the container; make sure it resolves outside the repo.
try:
    import concourse.bass as bass
except ImportError:  # pragma: no cover
    for _p in ("/opt/trn_rl_repo", "/root/.axon_site/_ro/trn_rl_repo"):
        if os.path.isdir(_p) and _p not in sys.path:
            sys.path.insert(0, _p)
    import concourse.bass as bass

import ml_dtypes
import concourse.tile as tile
import concourse.mybir as mybir
from concourse import bass_utils
from concourse.bass import ds

F32 = mybir.dt.float32
F32R = mybir.dt.float32r
BF16 = mybir.dt.bfloat16
AF = mybir.ActivationFunctionType

C = 1024          # model dim
H = 16            # heads
DH = 64           # head dim
NTOK = 1024       # tokens per batch (keys/values)
NQ = 512          # query tokens per core
KT = C // 128     # 8 feature tiles
HID = 4096
EPS = 1e-5

_cache = {}


# --------------------------------------------------------------------------
# Walrus on this image rejects instructions carrying more than one semaphore
# wait command (the Tile epilogue drain accumulates one per logical
# processor). Split the excess onto dedicated same-engine NOPs.
# --------------------------------------------------------------------------
def _split_wide_waits(nc, max_waits=1):
    ctr = 0
    for f in nc.m.functions:
        for b in f.blocks:
            out, changed = [], False
            for inst in b.instructions:
                si = getattr(inst, "sync_info", None)
                if si is not None and si.on_wait and len(si.on_wait) > max_waits:
                    waits = list(si.on_wait)
                    extra, keep = waits[:-max_waits], waits[-max_waits:]
                    for gs in range(0, len(extra), max_waits):
                        ctr += 1
                        nop = mybir.InstNoOp(
                            name=f"waitsplit-{ctr}", ins=[], outs=[])
                        nop.engine = inst.engine
                        nop.sync_info = mybir.SyncInfo(
                            on_wait=extra[gs:gs + max_waits], on_update=[])
                        out.append(nop)
                    inst.sync_info = mybir.SyncInfo(
                        on_wait=keep, on_update=list(si.on_update))
                    changed = True
                out.append(inst)
            if changed:
                b.instructions = out


def build_program(has_bias, gelu_func=None):
    """Build the single-core SPMD Bass program.

    has_bias: dict of bools for qk/v/proj/fc1/fc2 bias emission.
    gelu_func: override the MLP activation (CoreSim lacks Gelu).
    """
    nc = bass.Bass()

    xT = nc.dram_tensor("xT", [C, NTOK], F32, kind="ExternalInput")
    wq_m = nc.dram_tensor("wq_m", [KT, 128, C], BF16, kind="ExternalInput")
    wk_m = nc.dram_tensor("wk_m", [KT, 128, C], BF16, kind="ExternalInput")
    wv_r = nc.dram_tensor("wv_r", [KT, 128, C], BF16, kind="ExternalInput")
    wp_m = nc.dram_tensor("wp_m", [KT, 128, C], BF16, kind="ExternalInput")
    w1_m = nc.dram_tensor("w1_m", [HID // 128, 128, C], BF16, kind="ExternalInput")
    w2_m = nc.dram_tensor("w2_m", [KT, 128, HID], BF16, kind="ExternalInput")
    b_all = nc.dram_tensor("b_all", [1, 3 * C + C + HID + C], BF16,
                           kind="ExternalInput")
    yT = nc.dram_tensor("yT", [C, NQ], F32, kind="ExternalOutput")

    with tile.TileContext(nc) as tc:
        _emit(nc, tc, xT, wq_m, wk_m, wv_r, wp_m, w1_m, w2_m, b_all, yT,
              has_bias, gelu_func or AF.Gelu)
    return nc


def _emit(nc, tc, xT, wq_m, wk_m, wv_r, wp_m, w1_m, w2_m, b_all, yT,
          has_bias, gelu_func):
    pers = tc.alloc_tile_pool(name="pers", bufs=1)
    ones_c = pers.tile([128, 1], BF16, tag="ones_c")      # stats lhsT
    nc.vector.memset(ones_c, 1.0)
    ones_r16 = pers.tile([1, NQ], BF16, tag="ones_r16")   # bias rank-1 rhs
    nc.vector.memset(ones_r16, 1.0)
    ones_tok16 = pers.tile([1, 128], BF16, tag="ones_tok16")  # v-bias lhsT
    nc.vector.memset(ones_tok16, 1.0)
    eps_t = pers.tile([128, 1], F32, tag="eps_t")
    nc.vector.memset(eps_t, EPS)

    p_dram = tc.alloc_tile_pool(name="dscratch", bufs=4, space="DRAM")

    def bcast_rows(row_ap, rep_tile, nparts, ncols, nm):
        # SBUF [1, n] -> DRAM -> SBUF [nparts, n] replicated (DMA cannot
        # broadcast from an SBUF source: partition step would be 0).
        d = p_dram.tile([1, ncols], F32, tag="dscratch", name=nm)
        nc.sync.dma_start(d, row_ap)
        nc.sync.dma_start(rep_tile, d.to_broadcast((nparts, ncols)))

    any_bias = any(has_bias.values())
    if any_bias:
        bias_sb = pers.tile([1, 3 * C + C + HID + C], BF16, tag="bias_sb")
        nc.sync.dma_start(bias_sb, b_all[:])
        bq_of, bk_of, bv_of = 0, C, 2 * C
        bp_of, b1_of, b2_of = 3 * C, 4 * C, 4 * C + HID

    # x2 (attention residual) is read until the end; allocate it at the
    # bottom of the pool stack even though it is written only at proj time.
    p_x2 = tc.alloc_tile_pool(name="x2", bufs=KT)

    # ---------------- load x^T ----------------
    p_xT = tc.alloc_tile_pool(name="xT", bufs=KT)
    xt = []
    for k in range(KT):
        t = p_xT.tile([128, NTOK], F32, tag="xT")
        nc.sync.dma_start(t, xT[ds(k * 128, 128), :])
        xt.append(t)

    # ---------------- LN1 (feature-major) ----------------
    # mean / mean-of-squares via ones-matmuls (fp32r = full speed), then
    # h1 = x * rstd_rep - (mu * rstd)_rep, output bf16.
    p_h1 = tc.alloc_tile_pool(name="h1", bufs=KT)
    p_ln1 = tc.alloc_tile_pool(name="ln1", bufs=1)
    p_sq = tc.alloc_tile_pool(name="sq", bufs=3)
    ps_stat = tc.alloc_tile_pool(name="ps_stat", bufs=1, space="PSUM")

    # stats matmuls run in bf16 (walrus requires fp32r inputs to be
    # explicitly rounded; bf16 stat error ~1e-4, negligible here)
    ms = [ps_stat.tile([1, 512], F32, tag=f"ms{n}", name=f"ms{n}")
          for n in range(2)]
    ss = [ps_stat.tile([1, 512], F32, tag=f"ss{n}", name=f"ss{n}")
          for n in range(2)]
    for k in range(KT):
        xb = p_sq.tile([128, NTOK], BF16, tag="xb")
        nc.vector.tensor_copy(xb, xt[k])
        sq = p_sq.tile([128, NTOK], BF16, tag="sq")
        nc.scalar.activation(sq, xt[k], AF.Square)
        for n in range(2):
            nc.tensor.matmul(ms[n], ones_c, xb[:, ds(n * 512, 512)],
                             start=(k == 0), stop=(k == KT - 1))
            nc.tensor.matmul(ss[n], ones_c, sq[:, ds(n * 512, 512)],
                             start=(k == 0), stop=(k == KT - 1))
    p_sq.release()

    mu = p_ln1.tile([1, NTOK], F32, tag="mu")
    es = p_ln1.tile([1, NTOK], F32, tag="es")
    for n in range(2):
        nc.vector.tensor_scalar_mul(mu[:, ds(n * 512, 512)], ms[n], 1.0 / C)
        nc.vector.tensor_scalar_mul(es[:, ds(n * 512, 512)], ss[n], 1.0 / C)
    ps_stat.release()
    var = p_ln1.tile([1, NTOK], F32, tag="var")
    nc.vector.tensor_mul(var, mu, mu)                  # mu^2
    nc.vector.tensor_sub(var, es, var)                 # E[x^2] - mu^2
    std = p_ln1.tile([1, NTOK], F32, tag="std")
    nc.scalar.activation(std, var, AF.Sqrt, bias=eps_t[ds(0, 1), :])
    rstd = p_ln1.tile([1, NTOK], F32, tag="rstd")
    nc.vector.reciprocal(rstd, std)
    musc = p_ln1.tile([1, NTOK], F32, tag="musc")
    nc.vector.tensor_mul(musc, mu, rstd)               # mu * rstd
    rstd_rep = p_ln1.tile([128, NTOK], F32, tag="rstd_rep")
    musc_rep = p_ln1.tile([128, NTOK], F32, tag="musc_rep")
    bcast_rows(rstd, rstd_rep, 128, NTOK, "b_rstd")
    bcast_rows(musc, musc_rep, 128, NTOK, "b_musc")

    p_tmp = tc.alloc_tile_pool(name="tmp", bufs=3)
    h1 = []
    for k in range(KT):
        tmp = p_tmp.tile([128, NTOK], F32, tag="tmp")
        nc.vector.tensor_mul(tmp, xt[k], rstd_rep)
        h = p_h1.tile([128, NTOK], BF16, tag="h1")
        nc.vector.tensor_sub(h, tmp, musc_rep)
        h1.append(h)
    p_tmp.release()
    p_ln1.release()

    # ---------------- V (token-major, with ones column) ----------------
    p_V = tc.alloc_tile_pool(name="V", bufs=KT)
    p_wv = tc.alloc_tile_pool(name="wv", bufs=KT)
    ps_v = tc.alloc_tile_pool(name="ps_v", bufs=3, space="PSUM")

    wv = []
    for k in range(KT):
        t = p_wv.tile([128, C], BF16, tag="wv")
        nc.sync.dma_start(t, wv_r[k, :, :])
        wv.append(t)

    V = []   # per token-tile: [128, H, 65], col 64 = 1.0
    for t in range(KT):
        vt = p_V.tile([128, H, 65], BF16, tag="V")
        nc.vector.memset(vt[:, :, ds(64, 1)], 1.0)
        for n in range(2):
            ps = ps_v.tile([128, 512], F32, tag="ps_v")
            for k in range(KT):
                nc.tensor.matmul(
                    ps, h1[k][:, ds(t * 128, 128)], wv[k][:, ds(n * 512, 512)],
                    start=(k == 0), stop=(k == KT - 1 and not has_bias["v"]))
            if has_bias["v"]:
                nc.tensor.matmul(
                    ps, ones_tok16,
                    bias_sb[:, ds(bv_of + n * 512, 512)],
                    start=False, stop=True)
            # scatter the 8 heads of this 512-chunk into [*, h, 0:64]
            nc.vector.tensor_copy(
                vt[:, ds(n * 8, 8), ds(0, 64)],
                ps.rearrange("p (h d) -> p h d", d=64))
        V.append(vt)
    p_wv.release()
    ps_v.release()

    # ---------------- attention loop over head pairs ----------------
    # pair t = heads (2t, 2t+1); K^T/Q^T feature tile t holds both heads.
    p_O = tc.alloc_tile_pool(name="O", bufs=KT)
    p_K = tc.alloc_tile_pool(name="K", bufs=KT)
    p_Q = tc.alloc_tile_pool(name="Q", bufs=KT)
    p_P = tc.alloc_tile_pool(name="P", bufs=40)
    p_rq = tc.alloc_tile_pool(name="rq", bufs=4)
    p_rep = tc.alloc_tile_pool(name="rep", bufs=4)
    p_wkq = tc.alloc_tile_pool(name="wkq", bufs=4)
    ps_a = tc.alloc_tile_pool(name="ps_a", bufs=2, space="PSUM")
    ps_s = tc.alloc_tile_pool(name="ps_s", bufs=4, space="PSUM")
    ps_o = tc.alloc_tile_pool(name="ps_o", bufs=2, space="PSUM")

    K_sb, Q_sb, P_sb, O_sb = [], [], {}, []

    def emit_kq(t):
        wkt = p_wkq.tile([128, C], BF16, tag="wkq")
        nc.sync.dma_start(wkt, wk_m[t, :, :])
        kt_sb = p_K.tile([128, NTOK], BF16, tag="K")
        for n in range(2):
            ps = ps_a.tile([128, 512], F32, tag="ps_a")
            for k in range(KT):
                nc.tensor.matmul(
                    ps, wkt[:, ds(k * 128, 128)], h1[k][:, ds(n * 512, 512)],
                    start=(k == 0), stop=(k == KT - 1 and not has_bias["qk"]))
            if has_bias["qk"]:
                nc.tensor.matmul(
                    ps, bias_sb[:, ds(bk_of + t * 128, 128)], ones_r16,
                    start=False, stop=True)
            nc.vector.tensor_copy(kt_sb[:, ds(n * 512, 512)], ps)
        K_sb.append(kt_sb)

        wqt = p_wkq.tile([128, C], BF16, tag="wkq")
        nc.sync.dma_start(wqt, wq_m[t, :, :])
        qt_sb = p_Q.tile([128, NQ], BF16, tag="Q")
        ps = ps_a.tile([128, 512], F32, tag="ps_a")
        for k in range(KT):
            nc.tensor.matmul(
                ps, wqt[:, ds(k * 128, 128)], h1[k][:, ds(0, 512)],
                start=(k == 0), stop=(k == KT - 1 and not has_bias["qk"]))
        if has_bias["qk"]:
            nc.tensor.matmul(
                ps, bias_sb[:, ds(bq_of + t * 128, 128)], ones_r16,
                start=False, stop=True)
        nc.vector.tensor_copy(qt_sb, ps)
        Q_sb.append(qt_sb)

    def emit_st(t):
        # S^T then exp, per 128-key block m; two heads ride the PE array
        # concurrently on row strips [0:64] / [64:128].
        for m in range(KT):
            for h2 in range(2):
                lo = h2 * 64
                ps = ps_s.tile([128, 512], F32, tag="ps_s")
                nc.tensor.matmul(
                    ps,
                    K_sb[t][ds(lo, 64), ds(m * 128, 128)],
                    Q_sb[t][ds(lo, 64), :],
                    start=True, stop=True)
                p = p_P.tile([128, 512], BF16, tag="P")
                nc.scalar.activation(p, ps, AF.Exp, scale=float(DH) ** -0.5)
                P_sb[(t, h2, m)] = p

    def emit_pv(t):
        ot = p_O.tile([128, NQ], BF16, tag="O")
        for h2 in range(2):
            head = 2 * t + h2
            ps = ps_o.tile([128, 512], F32, tag="ps_o")
            for k in range(KT):
                nc.tensor.matmul(
                    ps[ds(0, 65), :],
                    V[k][:, head, :],
                    P_sb[(t, h2, k)],
                    start=(k == 0), stop=(k == KT - 1))
            rq = p_rq.tile([1, 512], F32, tag="rq")
            nc.vector.reciprocal(rq, ps[ds(64, 1), :])
            rep = p_rep.tile([64, 512], F32, tag="rep")
            bcast_rows(rq, rep, 64, 512, f"b_rq{t}_{h2}")
            nc.vector.tensor_mul(ot[ds(h2 * 64, 64), :], ps[ds(0, 64), :], rep)
        O_sb.append(ot)

    for t in range(KT):
        emit_kq(t)
        emit_st(t)
        if t >= 1:
            emit_pv(t - 1)
    emit_pv(KT - 1)
    for p in (p_wkq, p_rep, p_rq, p_P, p_Q, p_K):
        p.release()
    for p in (ps_o, ps_s, ps_a):
        p.release()

    # ---------------- proj + residual + LN2 stats ----------------
    p_wp = tc.alloc_tile_pool(name="wp", bufs=3)
    p_sq2 = tc.alloc_tile_pool(name="sq2", bufs=3)
    ps_st2 = tc.alloc_tile_pool(name="ps_st2", bufs=1, space="PSUM")
    ps_p = tc.alloc_tile_pool(name="ps_p", bufs=3, space="PSUM")

    ms2 = ps_st2.tile([1, 512], F32, tag="ms2")
    ss2 = ps_st2.tile([1, 512], F32, tag="ss2")
    x2 = []
    for m in range(KT):
        wpt = p_wp.tile([128, C], BF16, tag="wp")
        nc.sync.dma_start(wpt, wp_m[m, :, :])
        ps = ps_p.tile([128, 512], F32, tag="ps_p")
        for k in range(KT):
            nc.tensor.matmul(
                ps, wpt[:, ds(k * 128, 128)], O_sb[k],
                start=(k == 0), stop=(k == KT - 1 and not has_bias["proj"]))
        if has_bias["proj"]:
            nc.tensor.matmul(ps, bias_sb[:, ds(bp_of + m * 128, 128)],
                             ones_r16, start=False, stop=True)
        xm = p_x2.tile([128, NQ], F32, tag="x2")
        nc.vector.tensor_add(xm, ps, xt[m][:, ds(0, NQ)])
        x2.append(xm)
        xb2 = p_sq2.tile([128, NQ], BF16, tag="xb2")
        nc.vector.tensor_copy(xb2, xm)
        sq = p_sq2.tile([128, NQ], BF16, tag="sq2")
        nc.scalar.activation(sq, xm, AF.Square)
        nc.tensor.matmul(ms2, ones_c, xb2,
                         start=(m == 0), stop=(m == KT - 1))
        nc.tensor.matmul(ss2, ones_c, sq,
                         start=(m == 0), stop=(m == KT - 1))

    # release proj-phase + attention carry-over pools (space reused by MLP)
    for p in (p_sq2, p_wp, p_O, p_V, p_h1, p_xT):
        p.release()
    ps_p.release()

    # ---------------- LN2 ----------------
    p_ln2 = tc.alloc_tile_pool(name="ln2", bufs=1)
    mu2 = p_ln2.tile([1, NQ], F32, tag="mu2")
    es2 = p_ln2.tile([1, NQ], F32, tag="es2")
    nc.vector.tensor_scalar_mul(mu2, ms2, 1.0 / C)
    nc.vector.tensor_scalar_mul(es2, ss2, 1.0 / C)
    ps_st2.release()
    var2 = p_ln2.tile([1, NQ], F32, tag="var2")
    nc.vector.tensor_mul(var2, mu2, mu2)
    nc.vector.tensor_sub(var2, es2, var2)
    std2 = p_ln2.tile([1, NQ], F32, tag="std2")
    nc.scalar.activation(std2, var2, AF.Sqrt, bias=eps_t[ds(0, 1), :])
    rstd2 = p_ln2.tile([1, NQ], F32, tag="rstd2")
    nc.vector.reciprocal(rstd2, std2)
    musc2 = p_ln2.tile([1, NQ], F32, tag="musc2")
    nc.vector.tensor_mul(musc2, mu2, rstd2)
    rstd2_rep = p_ln2.tile([128, NQ], F32, tag="rstd2_rep")
    musc2_rep = p_ln2.tile([128, NQ], F32, tag="musc2_rep")
    bcast_rows(rstd2, rstd2_rep, 128, NQ, "b_rstd2")
    bcast_rows(musc2, musc2_rep, 128, NQ, "b_musc2")

    p_h2 = tc.alloc_tile_pool(name="h2", bufs=KT)
    p_tmp2 = tc.alloc_tile_pool(name="tmp2", bufs=3)
    h2t = []
    for k in range(KT):
        tmp = p_tmp2.tile([128, NQ], F32, tag="tmp2")
        nc.vector.tensor_mul(tmp, x2[k], rstd2_rep)
        h = p_h2.tile([128, NQ], BF16, tag="h2")
        nc.vector.tensor_sub(h, tmp, musc2_rep)
        h2t.append(h)
    p_tmp2.release()

    # ---------------- MLP ----------------
    p_g = tc.alloc_tile_pool(name="g", bufs=HID // 128)
    p_w1 = tc.alloc_tile_pool(name="w1", bufs=4)
    ps_m = tc.alloc_tile_pool(name="ps_m", bufs=4, space="PSUM")

    g_sb = []
    for m in range(HID // 128):
        w1t = p_w1.tile([128, C], BF16, tag="w1")
        nc.sync.dma_start(w1t, w1_m[m, :, :])
        ps = ps_m.tile([128, 512], F32, tag="ps_m")
        for k in range(KT):
            nc.tensor.matmul(
                ps, w1t[:, ds(k * 128, 128)], h2t[k],
                start=(k == 0), stop=(k == KT - 1 and not has_bias["fc1"]))
        if has_bias["fc1"]:
            nc.tensor.matmul(ps, bias_sb[:, ds(b1_of + m * 128, 128)],
                             ones_r16, start=False, stop=True)
        g = p_g.tile([128, NQ], BF16, tag="g")
        nc.scalar.activation(g, ps, gelu_func)
        g_sb.append(g)
    p_w1.release()

    p_w2 = tc.alloc_tile_pool(name="w2", bufs=3)
    p_y = tc.alloc_tile_pool(name="y", bufs=3)
    for m in range(KT):
        w2t = p_w2.tile([128, HID], BF16, tag="w2")
        nc.sync.dma_start(w2t, w2_m[m, :, :])
        ps = ps_m.tile([128, 512], F32, tag="ps_m")
        for k in range(HID // 128):
            nc.tensor.matmul(
                ps, w2t[:, ds(k * 128, 128)], g_sb[k],
                start=(k == 0), stop=(k == HID // 128 - 1 and not has_bias["fc2"]))
        if has_bias["fc2"]:
            nc.tensor.matmul(ps, bias_sb[:, ds(b2_of + m * 128, 128)],
                             ones_r16, start=False, stop=True)
        y = p_y.tile([128, NQ], F32, tag="y")
        nc.vector.tensor_add(y, ps, x2[m])
        nc.sync.dma_start(yT[ds(m * 128, 128), :], y)

    for p in (p_y, p_w2, p_g, p_h2, p_ln2, p_x2, pers):
        p.release()
    ps_m.release()
    p_dram.release()


# --------------------------------------------------------------------------
# Host side
# --------------------------------------------------------------------------
def _m_slice(w, mtiles):
    """[K_in, M_out] -> [mtiles, 128, K_in] with free dim k-major
    (arr[m, i, k*128+j] = w[k*128+i, m*128+j])."""
    kin = w.shape[0]
    kt = kin // 128
    a = w.reshape(kt, 128, mtiles, 128)        # [k, i, m, j]
    return np.ascontiguousarray(a.transpose(2, 1, 0, 3).reshape(mtiles, 128, kin))


def _prep(inputs):
    f32 = np.float32
    x = np.asarray(inputs["x"], f32)
    ln1_g = np.asarray(inputs["ln1_g"], f32)
    ln1_b = np.asarray(inputs["ln1_b"], f32)
    ln2_g = np.asarray(inputs["ln2_g"], f32)
    ln2_b = np.asarray(inputs["ln2_b"], f32)
    w_qkv = np.asarray(inputs["w_qkv"], f32)
    w_proj = np.asarray(inputs["w_proj"], f32)
    w_fc1 = np.asarray(inputs["w_fc1"], f32)
    w_fc2 = np.asarray(inputs["w_fc2"], f32)

    # fold LN affine params into the following matmul
    wqkv_e = ln1_g[:, None] * w_qkv
    bqkv_e = ln1_b @ w_qkv + np.asarray(inputs["b_qkv"], f32)
    wfc1_e = ln2_g[:, None] * w_fc1
    bfc1_e = ln2_b @ w_fc1 + np.asarray(inputs["b_fc1"], f32)
    b_proj = np.asarray(inputs["b_proj"], f32)
    b_fc2 = np.asarray(inputs["b_fc2"], f32)

    bf = ml_dtypes.bfloat16
    wq, wk, wvv = wqkv_e[:, :C], wqkv_e[:, C:2 * C], wqkv_e[:, 2 * C:]
    shared = {
        "wq_m": _m_slice(wq, KT).astype(bf),
        "wk_m": _m_slice(wk, KT).astype(bf),
        "wv_r": np.ascontiguousarray(wvv.reshape(KT, 128, C)).astype(bf),
        "wp_m": _m_slice(w_proj, KT).astype(bf),
        "w1_m": _m_slice(wfc1_e, HID // 128).astype(bf),
        "w2_m": _m_slice(w_fc2, KT).astype(bf),
        "b_all": np.concatenate(
            [bqkv_e, b_proj, bfc1_e, b_fc2])[None, :].astype(bf),
    }
    has_bias = {
        "qk": bool(np.any(bqkv_e[:2 * C])),
        "v": bool(np.any(bqkv_e[2 * C:])),
        "proj": bool(np.any(b_proj)),
        "fc1": bool(np.any(bfc1_e)),
        "fc2": bool(np.any(b_fc2)),
    }

    in_maps = []
    for c in range(8):
        b, half = c // 2, c % 2
        xb = x[b]
        if half:
            xb = np.concatenate([xb[NQ:], xb[:NQ]], axis=0)
        m = {"xT": np.ascontiguousarray(xb.T), **shared}
        in_maps.append(m)
    return in_maps, has_bias


def kernel(**inputs):
    in_maps, has_bias = _prep(inputs)
    key = tuple(sorted(has_bias.items()))
    if key not in _cache:
        nc = build_program(has_bias)
        _split_wide_waits(nc, 1)
        _cache[key] = nc
    nc = _cache[key]

    res = bass_utils.run_bass_kernel_spmd(
        nc, in_maps, core_ids=list(range(8)), trace=False)

    x = np.asarray(inputs["x"])
    out = np.empty((4, NTOK, C), dtype=np.float32)
    for c in range(8):
        b, half = c // 2, c % 2
        out[b, half * NQ:(half + 1) * NQ, :] = res.results[c]["yT"].T
    return out.astype(x.dtype, copy=False)
